# revision 17
# baseline (speedup 1.0000x reference)
"""2D DCT-II (4096x4096, f32) on 8 Trainium2 NeuronCores.

out = Cm @ x @ Cn^T with Cm[u,i] = cos(pi*(2i+1)*u/(2M)) — mathematically
identical to the reference's Makhoul-FFT formulation.

Both passes exploit the exact DCT symmetry C[u, N-1-i] = (-1)^u C[u, i],
which halves each contraction:
  pass 1: g/h = x_top ± reversed(x_bot), A^T[c,t] = sum_{i<2048} fold[i,c]*cmt[i,t]
  pass 2: g2/h2 = A^T_top ± reversed(A^T_bot), out = folds @ [cnt_e | cnt_o]

Sharding: cores 0-3 compute even output rows u=2t (they get the + fold via a
host-permuted operand), cores 4-7 odd rows u=2t+1 (host-permuted operand is
negated so the same on-device ADD graph computes the - fold). Pass 2's fold
needs a cross-partition reversal of SBUF-resident A^T tiles; that is done
with a 128x128 reversal-permutation matmul (J @ tile) on the TensorEngine.
Output v-columns are produced parity-split ([even | odd]); the host
interleaves rows/columns when assembling the final array (pure permutation).
"""

import sys

for _p in ("/opt/trn_rl_repo", "/opt/pypackages"):
    if _p not in sys.path:
        sys.path.append(_p)

import numpy as np

M = 4096
N = 4096
H = M // 2          # 2048: folded contraction length
N_CORES = 8
TS = 512            # t-shard width per core (512 outputs rows per core)

_CACHE = {}


def _build_nc():
    import concourse.bacc as bacc
    import concourse.mybir as mybir
    from concourse import tile

    BF16 = mybir.dt.bfloat16
    F32 = mybir.dt.float32

    nc = bacc.Bacc("TRN2", target_bir_lowering=False, debug=False,
                   num_devices=N_CORES)
    xa = nc.dram_tensor("xa", [H, N], BF16, kind="ExternalInput")
    xb = nc.dram_tensor("xb", [H, N], BF16, kind="ExternalInput")
    cmt = nc.dram_tensor("cmt", [H, TS], BF16, kind="ExternalInput")
    cnt = nc.dram_tensor("cnt", [H, N], BF16, kind="ExternalInput")
    jrev = nc.dram_tensor("jrev", [128, 128], BF16, kind="ExternalInput")
    out = nc.dram_tensor("out", [TS, N], F32, kind="ExternalOutput")

    with tile.TileContext(nc) as tc:
        with (
            tc.tile_pool(name="persist", bufs=1) as persist,
            tc.tile_pool(name="stream", bufs=4) as stream,
            tc.tile_pool(name="psum", bufs=8, space="PSUM") as pp,
        ):
            jt = persist.tile([128, 128], BF16, name="jt")

            cmt_sb = [persist.tile([128, TS], BF16, tag=f"cmt{j}",
                                   name=f"cmt_sb{j}")
                      for j in range(16)]

            a_sb = [persist.tile([128, TS], BF16, tag=f"a{cc}",
                                 name=f"a_sb{cc}")
                    for cc in range(32)]
            g2 = [persist.tile([128, TS], BF16, tag=f"g2_{cc}",
                               name=f"g2_{cc}")
                  for cc in range(16)]
            h2 = [persist.tile([128, TS], BF16, tag=f"h2_{cc}",
                               name=f"h2_{cc}")
                  for cc in range(16)]

            # ---- pass 1: A^T[c, t] = sum_{i<H} fold[i, c] * cmt[i, t]
            for cg in range(4):          # 1024-wide c-groups
                ps = [pp.tile([128, TS], F32, tag="ps", name=f"ps1_{cg}_{i}")
                      for i in range(8)]
                for j in range(16):      # contraction chunks over i
                    xt = stream.tile([128, 1024], BF16, tag="xt")
                    xr = stream.tile([128, 1024], BF16, tag="xr")
                    nc.sync.dma_start(
                        xt[:], xa[j * 128:(j + 1) * 128,
                                  cg * 1024:(cg + 1) * 1024])
                    nc.sync.dma_start(
                        xr[:], xb[j * 128:(j + 1) * 128,
                                  cg * 1024:(cg + 1) * 1024])
                    if cg == 0:
                        # lazy constant loads: first x tiles aren't stuck
                        # behind a bulk preload at kernel start
                        nc.sync.dma_start(cmt_sb[j][:],
                                          cmt[j * 128:(j + 1) * 128, :])
                        if j == 0:
                            nc.sync.dma_start(jt[:], jrev[:])
                    gj = stream.tile([128, 1024], BF16, tag="gj")
                    nc.vector.tensor_add(gj[:], xt[:], xr[:])
                    for cs in range(8):
                        nc.tensor.matmul(
                            ps[cs][:],
                            gj[:, cs * 128:(cs + 1) * 128],
                            cmt_sb[j][:],
                            start=(j == 0), stop=(j == 15))
                for cs in range(8):
                    if cs % 2 == 0:
                        nc.vector.tensor_copy(a_sb[cg * 8 + cs][:], ps[cs][:])
                    else:
                        nc.scalar.copy(a_sb[cg * 8 + cs][:], ps[cs][:])

            # ---- pass 2 fold: g2/h2[c,t] = A^T[c,t] +/- A^T[M-1-c,t]
            for cc in range(16):
                rev = pp.tile([128, TS], F32, tag="ps", name=f"rev{cc}")
                nc.tensor.matmul(rev[:], jt[:], a_sb[31 - cc][:],
                                 start=True, stop=True)
                nc.vector.tensor_add(g2[cc][:], a_sb[cc][:], rev[:])
                nc.vector.tensor_sub(h2[cc][:], a_sb[cc][:], rev[:])

            # ---- pass 2: out[t, 2s] = sum_c g2[c,t] cnt_e[c,s]
            #              out[t, 2s+1] = sum_c h2[c,t] cnt_o[c,s]
            for sg in range(4):          # 512-wide s-groups
                pe = [pp.tile([128, 512], F32, tag="ps", name=f"pe_{sg}_{i}")
                      for i in range(4)]
                po = [pp.tile([128, 512], F32, tag="ps", name=f"po_{sg}_{i}")
                      for i in range(4)]
                for cc in range(16):     # contraction chunks over c
                    # cnt is host-packed so [even_sg | odd_sg] is one
                    # contiguous 1024-block -> single 2KB-line DMA
                    ct = stream.tile([128, 1024], BF16, tag="ct")
                    nc.sync.dma_start(
                        ct[:], cnt[cc * 128:(cc + 1) * 128,
                                   sg * 1024:(sg + 1) * 1024])
                    for us in range(4):
                        nc.tensor.matmul(
                            pe[us][:],
                            g2[cc][:, us * 128:(us + 1) * 128],
                            ct[:, 0:512],
                            start=(cc == 0), stop=(cc == 15))
                        nc.tensor.matmul(
                            po[us][:],
                            h2[cc][:, us * 128:(us + 1) * 128],
                            ct[:, 512:1024],
                            start=(cc == 0), stop=(cc == 15))
                for us in range(4):
                    # stage PSUM -> SBUF on both Vector and Scalar engines,
                    # then one 1024-wide store (out cols sg-block packed)
                    ot = stream.tile([128, 1024], F32, tag="ot")
                    nc.vector.tensor_copy(ot[:, 0:512], pe[us][:])
                    nc.scalar.copy(ot[:, 512:1024], po[us][:])
                    # stores go on the GpSimd (SWDGE) queue so they never
                    # block the next s-group's rhs loads on the Sync queue
                    nc.gpsimd.dma_start(
                        out[us * 128:(us + 1) * 128,
                            sg * 1024:(sg + 1) * 1024], ot[:])
    nc.finalize()
    return nc


def _consts():
    """Host-precomputed constant operands (input-independent)."""
    import ml_dtypes
    bf16 = ml_dtypes.bfloat16
    i = np.arange(H, dtype=np.float64)[:, None]
    t = np.arange(H, dtype=np.float64)[None, :]
    ce = np.cos(np.pi * (2 * i + 1) * (2 * t) / (2 * M))       # [i<H, t<H]
    co = np.cos(np.pi * (2 * i + 1) * (2 * t + 1) / (2 * M))
    # pack per 512-wide s-group: [e_sg0 | o_sg0 | e_sg1 | o_sg1 ...] so each
    # pass-2 rhs tile is one contiguous 1024-col (2KB-line) DMA
    blocks = []
    for sg in range(4):
        blocks.append(ce[:, sg * 512:(sg + 1) * 512])
        blocks.append(co[:, sg * 512:(sg + 1) * 512])
    cnt = np.ascontiguousarray(
        np.concatenate(blocks, axis=1).astype(bf16))            # [H, N]
    cmt_e = ce.astype(bf16)                                     # [H, H]
    cmt_o = co.astype(bf16)
    jrev = np.zeros((128, 128), dtype=bf16)
    jrev[np.arange(128), 127 - np.arange(128)] = 1
    return cmt_e, cmt_o, cnt, jrev


def _run_res(x_np, trace=False):
    from concourse.bass_utils import run_bass_kernel_spmd
    import ml_dtypes
    bf16 = ml_dtypes.bfloat16

    if "nc" not in _CACHE:
        _CACHE["nc"] = _build_nc()
        _CACHE["consts"] = _consts()
    nc = _CACHE["nc"]
    cmt_e, cmt_o, cnt, jrev = _CACHE["consts"]

    x_np = np.asarray(x_np, dtype=np.float32)
    xa = np.ascontiguousarray(x_np[:H].astype(bf16))
    xb_pos = np.ascontiguousarray(x_np[M - 1:H - 1:-1].astype(bf16))
    xb_neg = np.ascontiguousarray(-x_np[M - 1:H - 1:-1].astype(bf16))

    in_maps = []
    for k in range(N_CORES):
        par = 0 if k < 4 else 1
        ksh = k % 4
        cm = cmt_e if par == 0 else cmt_o
        in_maps.append({
            "xa": xa,
            "xb": xb_pos if par == 0 else xb_neg,
            "cmt": np.ascontiguousarray(cm[:, ksh * TS:(ksh + 1) * TS]),
            "cnt": cnt,
            "jrev": jrev,
        })
    res = run_bass_kernel_spmd(nc, in_maps, core_ids=list(range(N_CORES)),
                               trace=trace)

    out = np.empty((M, N), dtype=np.float32)
    for k in range(N_CORES):
        r = res.results[k]["out"]    # [TS, N] cols packed [e_sg|o_sg] blocks
        par = 0 if k < 4 else 1
        t0 = (k % 4) * TS
        rows = slice(2 * t0 + par, 2 * (t0 + TS) + par, 2)
        out[rows] = (r.reshape(TS, 4, 2, 512)
                      .transpose(0, 1, 3, 2).reshape(TS, N))
    return out, res.exec_time_ns, res


def kernel(x):
    out, _, _ = _run_res(np.asarray(x), trace=False)
    return out
